# revision 13
# baseline (speedup 1.0000x reference)
"""MoE (16 routed experts, top-2, + shared expert) on 8 Trainium2 cores.

Strategy (expert-parallel, host-side dispatch):
  - Host computes the gate (softmax + top-2) and gathers each expert's
    tokens; core c owns experts 2c and 2c+1.
  - The shared expert is data-parallel: core c processes tokens
    [c*T/8, (c+1)*T/8).
  - Each core runs the same program: three gated-MLP "blocks"
    (expert0, expert1, shared) in a transposed layout
        zT = W2^T @ (u * silu(g)),  [u;g]^T = W1^T @ xT
    so no on-chip transposes are needed anywhere.
  - Host pre-arranges weights per-core into the exact SBUF tile layout so
    every DMA reads 4KB-contiguous runs per partition (full HBM rate).
  - Host applies the top-2 combine weights and scatter-adds expert
    outputs, then adds the shared-expert outputs.

Matmuls run as fp32r (full PE rate at free-dim >= 256, ~3e-4 rel err).
"""

import sys

for _p in ("/opt/trn_rl_repo", "/root/.axon_site/_ro/trn_rl_repo"):
    if _p not in sys.path:
        sys.path.insert(0, _p)

import contextlib

import numpy as np

import concourse.bass as bass  # noqa: F401
import concourse.tile as tile
from concourse import bacc, mybir
from concourse.bass_utils import run_bass_kernel_spmd

B, S, D = 2, 1024, 1024
H = 512           # routed expert hidden
HS = 1024         # shared expert hidden
E = 16
ROUTE_SCALE = 1.0
T = B * S
N_CORES = 8
EPC = E // N_CORES          # experts per core
TDP = T // N_CORES          # shared-expert tokens per core
P = 128
KD = D // P                 # fc1 contraction chunks
MD = D // P                 # fc2 output chunks

DTYPE = "f16"          # "f32r" | "bf16" | "f16"
F32 = mybir.dt.float32
if DTYPE == "f32r":
    FPR, NPT = mybir.dt.float32r, np.float32
elif DTYPE == "bf16":
    import ml_dtypes
    FPR, NPT = mybir.dt.bfloat16, ml_dtypes.bfloat16
else:
    FPR, NPT = mybir.dt.float16, np.float16
ACT = mybir.ActivationFunctionType

LAST_RESULTS = None
_NC_CACHE = {}


def _build_nc(CR, reps=1):
    """SPMD program: two routed-expert blocks (capacity CR) + shared block.

    Weight params arrive pre-arranged:
      w1: [2*NH, P, KD*P]   chunk j = hc*2 + (0:u | 1:g), 4KB runs
      w2: [MD, P, NH*P]
      x:  [P, KD*C]
    reps > 1 wraps the body in a dynamic loop (benchmarking only)."""
    nc = bacc.Bacc(None, target_bir_lowering=False)

    blocks = []
    for i in range(EPC):
        NH = H // P
        blocks.append((
            nc.declare_dram_parameter(f"w1e{i}", [2 * NH, P, KD * P], FPR, isOutput=False),
            nc.declare_dram_parameter(f"w2e{i}", [MD, P, NH * P], FPR, isOutput=False),
            H,
            nc.declare_dram_parameter(f"xg{i}", [P, KD * CR], FPR, isOutput=False),
            CR,
            nc.declare_dram_parameter(f"zg{i}", [D, CR], F32, isOutput=True),
        ))
    NHS = HS // P
    blocks.append((
        nc.declare_dram_parameter("ws1", [2 * NHS, P, KD * P], FPR, isOutput=False),
        nc.declare_dram_parameter("ws2", [MD, P, NHS * P], FPR, isOutput=False),
        HS,
        nc.declare_dram_parameter("xd", [P, KD * TDP], FPR, isOutput=False),
        TDP,
        nc.declare_dram_parameter("zs", [D, TDP], F32, isOutput=True),
    ))

    with tile.TileContext(nc) as tc:
        with (
            tc.tile_pool(name="xpool", bufs=2) as xpool,
            tc.tile_pool(name="w1pool", bufs=8) as w1pool,
            tc.tile_pool(name="w2pool", bufs=6) as w2pool,
            tc.tile_pool(name="hpool", bufs=2) as hpool,
            tc.tile_pool(name="spool", bufs=4) as spool,
            tc.tile_pool(name="opool", bufs=4) as opool,
            tc.tile_pool(name="psu", bufs=3, space="PSUM") as psu,
            tc.tile_pool(name="psg", bufs=3, space="PSUM") as psg,
            tc.tile_pool(name="psz", bufs=2, space="PSUM") as psz,
        ):
            pools = (xpool, w1pool, w2pool, hpool, spool, opool, psu, psg, psz)
            loop_cm = tc.For_i(0, reps, 1) if reps > 1 else contextlib.nullcontext()
            with loop_cm:
                _emit_body(nc, blocks, pools)
    nc.finalize()
    return nc


def _emit_body(nc, blocks, pools):
    xpool, w1pool, w2pool, hpool, spool, opool, psu, psg, psz = pools
    for w1, w2, HB, xt, C, zt in blocks:
        NH = HB // P
        w1_a = w1.ap()
        w2_a = w2.ap()
        xt_a = xt.ap().rearrange("p (ko c) -> p ko c", ko=KD)
        zt_a = zt.ap().rearrange("(mo mi) c -> mi mo c", mi=P)

        x_tile = xpool.tile([P, KD, C], FPR, tag=f"x{C}")
        nc.sync.dma_start(x_tile[:], xt_a)
        h_tile = hpool.tile([P, NH, C], FPR, tag=f"h{NH}_{C}")

        for hc in range(NH):
            w1t = w1pool.tile([P, 2, KD, P], FPR, tag="w1")
            nc.sync.dma_start(
                w1t[:],
                w1_a[2 * hc:2 * hc + 2].rearrange("s p (ko f) -> p s ko f", ko=KD))
            ps_u = psu.tile([P, C], F32, tag="psu")
            ps_g = psg.tile([P, C], F32, tag="psg")
            for k in range(KD):
                nc.tensor.matmul(ps_u[:], w1t[:, 0, k], x_tile[:, k],
                                 start=(k == 0), stop=(k == KD - 1))
            for k in range(KD):
                nc.tensor.matmul(ps_g[:], w1t[:, 1, k], x_tile[:, k],
                                 start=(k == 0), stop=(k == KD - 1))
            sil = spool.tile([P, C], F32, tag="sil")
            nc.scalar.activation(sil[:], ps_g[:], ACT.Silu)
            nc.vector.tensor_mul(h_tile[:, hc], ps_u[:], sil[:])

        for dp in range(MD // 2):
            w2t = w2pool.tile([P, 2, NH, P], FPR, tag=f"w2{NH}")
            nc.scalar.dma_start(
                w2t[:],
                w2_a[2 * dp:2 * dp + 2].rearrange("s p (ko f) -> p s ko f", ko=NH))
            for s2 in range(2):
                ps_z = psz.tile([P, C], F32, tag="psz")
                for k in range(NH):
                    nc.tensor.matmul(ps_z[:], w2t[:, s2, k], h_tile[:, k],
                                     start=(k == 0), stop=(k == NH - 1))
                o_tile = opool.tile([P, C], F32, tag="o")
                nc.vector.tensor_copy(o_tile[:], ps_z[:])
                nc.scalar.dma_start(zt_a[:, 2 * dp + s2], o_tile[:])


def _route(xf, Wg):
    """Host gate: softmax over expert logits, top-2 (ties -> lower index,
    matching jax.lax.top_k)."""
    logits = xf @ Wg.T
    m = logits.max(axis=-1, keepdims=True)
    p = np.exp(logits - m)
    scores = p / p.sum(axis=-1, keepdims=True)
    i1 = scores.argmax(axis=-1)
    rows = np.arange(T)
    s1 = scores[rows, i1]
    masked = scores.copy()
    masked[rows, i1] = -np.inf
    i2 = masked.argmax(axis=-1)
    s2 = scores[rows, i2]
    return i1, s1 * ROUTE_SCALE, i2, s2 * ROUTE_SCALE


def _pack_w1(W1b, HB):
    """[D, 2*HB] -> [2*NH, P, KD*P], chunk j = hc*2 + half, contiguous runs."""
    NH = HB // P
    Ar = W1b.reshape(KD, P, 2, NH, P)              # [ko, ki, half, hc, f]
    return np.ascontiguousarray(
        Ar.transpose(3, 2, 1, 0, 4).reshape(2 * NH, P, KD * P).astype(NPT))


def _pack_w2(W2b, HB):
    """[HB, D] -> [MD, P, NH*P]."""
    NH = HB // P
    Br = W2b.reshape(NH, P, MD, P)                 # [ko, ki, dc, f]
    return np.ascontiguousarray(
        Br.transpose(2, 1, 0, 3).reshape(MD, P, NH * P).astype(NPT))


def _pack_x(xTb, C):
    """[D, C] -> [P, KD*C] (zero-pads the token dim to C)."""
    n = xTb.shape[1]
    out = np.zeros((P, KD * C), dtype=NPT)
    out.reshape(P, KD, C)[:, :, :n] = xTb.reshape(KD, P, n).transpose(1, 0, 2)
    return out


def prepare(x, Wg, W1, W2, Ws1, Ws2):
    """Host routing + per-core input maps. Returns (in_maps, toks, wts, CR)."""
    x = np.asarray(x, dtype=np.float32)
    Wg = np.asarray(Wg, dtype=np.float32)
    W1 = np.asarray(W1, dtype=np.float32)
    W2 = np.asarray(W2, dtype=np.float32)
    Ws1 = np.asarray(Ws1, dtype=np.float32)
    Ws2 = np.asarray(Ws2, dtype=np.float32)

    xf = np.ascontiguousarray(x.reshape(T, D))
    i1, s1, i2, s2 = _route(xf, Wg)

    toks, wts = [], []
    for e in range(E):
        sel = np.where((i1 == e) | (i2 == e))[0]
        toks.append(sel)
        wts.append(np.where(i1[sel] == e, s1[sel], s2[sel]).astype(np.float32))

    max_n = max(len(t) for t in toks)
    CR = max(256, -(-max_n // 32) * 32)

    ws1p = _pack_w1(Ws1, HS)
    ws2p = _pack_w2(Ws2, HS)
    in_maps = []
    for c in range(N_CORES):
        im = {"ws1": ws1p, "ws2": ws2p,
              "xd": _pack_x(np.ascontiguousarray(xf[c * TDP:(c + 1) * TDP].T), TDP)}
        for i in range(EPC):
            e = EPC * c + i
            im[f"w1e{i}"] = _pack_w1(W1[e], H)
            im[f"w2e{i}"] = _pack_w2(W2[e], H)
            im[f"xg{i}"] = _pack_x(xf[toks[e]].T, CR)
        in_maps.append(im)
    return in_maps, toks, wts, CR


def kernel(x, Wg, W1, W2, Ws1, Ws2):
    global LAST_RESULTS
    in_maps, toks, wts, CR = prepare(x, Wg, W1, W2, Ws1, Ws2)

    if CR not in _NC_CACHE:
        _NC_CACHE[CR] = _build_nc(CR)
    nc = _NC_CACHE[CR]

    LAST_RESULTS = run_bass_kernel_spmd(nc, in_maps, list(range(N_CORES)))
    res = LAST_RESULTS.results

    out = np.zeros((T, D), dtype=np.float32)
    for c in range(N_CORES):
        for i in range(EPC):
            e = EPC * c + i
            n = len(toks[e])
            out[toks[e]] += wts[e][:, None] * res[c][f"zg{i}"][:, :n].T
        out[c * TDP:(c + 1) * TDP] += res[c]["zs"].T
    return out.reshape(B, S, D)
